# revision 5
# baseline (speedup 1.0000x reference)
"""Two-layer GCN (PyG GCNConv semantics) on 8 Trainium2 NeuronCores.

Strategy (sharding_hint): nodes are sharded row-wise across the 8 cores;
edges are partitioned by destination node so the segment-sum stays local;
source-node features are exchanged with an on-device AllGather between
layers; the small weight matrices are replicated.

Device pipeline per core (single NEFF, statically unrolled, SPMD-uniform):
  1. h1pre = x @ W1 for the local node shard (PE), written to DRAM.
  2. AllGather h1pre -> full gather table [N, HID].
  3. Per 128-node destination window: indirect-DMA gather of source rows,
     build a norm-weighted one-hot selection matrix on DVE
     (S[e, j] = norm[e] * (dst_rel[e] == j)), and accumulate
     msgs^T @ S into PSUM (PE).  Epilogue: relu(. + b1) (ACT) into an
     SBUF-resident transposed activation accumulator.
  4. h2pre = relu_out @ W2 (PE), AllGather again.
  5. Second gather/aggregate pass (S as lhsT so nodes land on partitions),
     then bias + log_softmax (DVE reduce + ACT exp/ln) and DMA out.

Edge bookkeeping (sorting by window, slot assignment, padding so that all
8 cores share one instruction stream) is host-side numpy index work; all
floating-point math on features runs on device.
"""

import math

import numpy as np

import concourse.bass as bass
import concourse.mybir as mybir
import concourse.tile as tile
from concourse.bass import IndirectOffsetOnAxis
from concourse.bass_utils import run_bass_kernel_spmd

N_NODES = 100000
N_EDGES = 1600000
IN_DIM, HID_DIM, OUT_DIM = 128, 64, 40
N_CORES = 8

F32 = mybir.dt.float32
I32 = mybir.dt.int32


def _split_long_waits(nc, max_waits=1):
    """This toolchain's codegen rejects instructions carrying more than one
    semaphore wait; move extra waits onto preceding same-engine no-ops."""
    cnt = 0
    for bb in nc.main_func.blocks:
        i = 0
        insts = bb.instructions
        while i < len(insts):
            ins = insts[i]
            si = ins.sync_info
            if si is not None and si.on_wait and len(si.on_wait) > max_waits:
                waits = list(si.on_wait)
                keep = waits[-max_waits:]
                extra = waits[:-max_waits]
                si.on_wait = keep
                new_insts = []
                for j in range(0, len(extra), max_waits):
                    chunk = extra[j : j + max_waits]
                    nop = mybir.InstNoOp(
                        name=f"{ins.name}-waitsplit-{j}",
                        engine=ins.engine,
                        ins=[],
                        outs=[],
                        sync_info=mybir.SyncInfo(on_wait=chunk, on_update=[]),
                    )
                    new_insts.append(nop)
                insts[i:i] = new_insts
                i += len(new_insts)
                cnt += len(new_insts)
            i += 1
    return cnt


def _preprocess(edge_index, n_nodes, n_cores):
    """Host-side index bookkeeping. Returns per-core slot arrays + layout."""
    nloc = n_nodes // n_cores
    wn = math.ceil(nloc / 128)

    src = np.asarray(edge_index[0], dtype=np.int64)
    dst = np.asarray(edge_index[1], dtype=np.int64)
    loop = np.arange(n_nodes, dtype=np.int64)
    src_all = np.concatenate([src, loop])
    dst_all = np.concatenate([dst, loop])

    deg = np.bincount(dst_all, minlength=n_nodes).astype(np.float64)
    dis = np.where(deg > 0, 1.0 / np.sqrt(deg), 0.0)
    norm = (dis[src_all] * dis[dst_all]).astype(np.float32)

    core = dst_all // nloc
    dloc = dst_all - core * nloc
    w = dloc >> 7
    dst_rel = (dloc & 127).astype(np.float32)

    key = core * wn + w
    order = np.argsort(key, kind="stable")
    counts = np.bincount(key, minlength=n_cores * wn).reshape(n_cores, wn)

    k_w = np.maximum((counts + 127) // 128, 1).max(axis=0)  # tiles per window
    t_total = int(k_w.sum())
    tile_base = np.zeros(wn, dtype=np.int64)
    tile_base[1:] = np.cumsum(k_w)[:-1]

    # rank of each edge within its (core, window) group, in sorted order
    grp_start = np.zeros(n_cores * wn, dtype=np.int64)
    grp_start[1:] = np.cumsum(counts.reshape(-1))[:-1]
    rank = np.arange(len(order), dtype=np.int64) - grp_start[key[order]]

    slot = (tile_base[w[order]] * 128 + rank).astype(np.int64)

    n_slots = t_total * 128
    idx_a = np.full((n_cores, n_slots), n_nodes, dtype=np.int32)
    drel_a = np.zeros((n_cores, n_slots), dtype=np.float32)
    norm_a = np.zeros((n_cores, n_slots), dtype=np.float32)

    c_o = core[order]
    idx_a[c_o, slot] = src_all[order].astype(np.int32)
    drel_a[c_o, slot] = dst_rel[order]
    norm_a[c_o, slot] = norm[order]

    # device layout: [128, T] with slot s at (s % 128, s // 128)
    idx_dev = [idx_a[c].reshape(t_total, 128).T.copy() for c in range(n_cores)]
    drel_dev = [drel_a[c].reshape(t_total, 128).T.copy() for c in range(n_cores)]
    norm_dev = [norm_a[c].reshape(t_total, 128).T.copy() for c in range(n_cores)]

    return {
        "nloc": nloc,
        "wn": wn,
        "k_w": [int(v) for v in k_w],
        "t_total": t_total,
        "idx": idx_dev,
        "drel": drel_dev,
        "norm": norm_dev,
    }


def _build_nc(n_nodes, nloc, wn, k_w, t_total, hid, out_dim, in_dim, n_cores):
    nc = bass.Bass(num_devices=n_cores)

    xT = nc.dram_tensor("xT", [in_dim, nloc], F32, kind="ExternalInput")
    idx = nc.dram_tensor("idx", [128, t_total], I32, kind="ExternalInput")
    drel = nc.dram_tensor("drel", [128, t_total], F32, kind="ExternalInput")
    nrm = nc.dram_tensor("nrm", [128, t_total], F32, kind="ExternalInput")
    w1 = nc.dram_tensor("w1", [in_dim, hid], F32, kind="ExternalInput")
    w2p = nc.dram_tensor("w2p", [hid, hid], F32, kind="ExternalInput")
    b1c = nc.dram_tensor("b1c", [hid, 1], F32, kind="ExternalInput")
    b2rep = nc.dram_tensor("b2rep", [128, hid], F32, kind="ExternalInput")
    iota_in = nc.dram_tensor("iota", [128, 128], F32, kind="ExternalInput")
    out = nc.dram_tensor("out", [nloc, out_dim], F32, kind="ExternalOutput")

    nb = math.ceil(nloc / 128)
    eq = mybir.AluOpType.is_equal
    mul = mybir.AluOpType.mult
    sub = mybir.AluOpType.subtract

    with tile.TileContext(nc) as tc:
        with (
            tc.tile_pool(name="const", bufs=1) as cp,
            tc.tile_pool(name="gpool", bufs=12) as gp,
            tc.tile_pool(name="spool", bufs=12) as sp,
            tc.tile_pool(name="evac", bufs=4) as ep,
            tc.tile_pool(name="ps_agg", bufs=4, space="PSUM") as pa,
            tc.tile_pool(name="ps_mm", bufs=4, space="PSUM") as pm,
            tc.tile_pool(name="dram", bufs=1, space="DRAM") as dp,
        ):
            # ---- resident tensors ----
            xT_t = cp.tile([in_dim, nloc], F32)
            nc.sync.dma_start(out=xT_t[:], in_=xT[:])
            idx_t = cp.tile([128, t_total], I32)
            nc.sync.dma_start(out=idx_t[:], in_=idx[:])
            drel_t = cp.tile([128, t_total], F32)
            nc.sync.dma_start(out=drel_t[:], in_=drel[:])
            nrm_t = cp.tile([128, t_total], F32)
            nc.sync.dma_start(out=nrm_t[:], in_=nrm[:])
            w1_t = cp.tile([in_dim, hid], F32)
            nc.sync.dma_start(out=w1_t[:], in_=w1[:])
            w2_t = cp.tile([hid, hid], F32)
            nc.sync.dma_start(out=w2_t[:], in_=w2p[:])
            b1_t = cp.tile([hid, 1], F32)
            nc.sync.dma_start(out=b1_t[:], in_=b1c[:])
            b2_t = cp.tile([128, hid], F32)
            nc.sync.dma_start(out=b2_t[:], in_=b2rep[:])
            iota_t = cp.tile([128, 128], F32)
            nc.sync.dma_start(out=iota_t[:], in_=iota_in[:])
            acc1T = cp.tile([hid, wn * 128], F32)
            zrow = cp.tile([128, hid], F32)
            nc.vector.memset(zrow[:], 0.0)

            h1loc = dp.tile([nloc, hid], F32)
            h1full = dp.tile([n_nodes + 128, hid], F32)
            h2loc = dp.tile([nloc, hid], F32)
            h2full = dp.tile([n_nodes + 128, hid], F32)

            # ---- 1. h1pre = x @ W1 (local shard) ----
            for b in range(nb):
                cols = min(128, nloc - b * 128)
                ps = pm.tile([128, hid], F32, tag="pmm")
                nc.tensor.matmul(
                    out=ps[:cols, :],
                    lhsT=xT_t[:, b * 128 : b * 128 + cols],
                    rhs=w1_t[:],
                    start=True,
                    stop=True,
                )
                hb = ep.tile([128, hid], F32, tag="hb")
                nc.vector.tensor_copy(out=hb[:cols, :], in_=ps[:cols, :])
                nc.sync.dma_start(
                    out=h1loc[b * 128 : b * 128 + cols, :], in_=hb[:cols, :]
                )

            # ---- 2. AllGather + zero pad row ----
            nc.gpsimd.collective_compute(
                "AllGather",
                mybir.AluOpType.bypass,
                replica_groups=[list(range(n_cores))],
                ins=[h1loc[:].opt()],
                outs=[h1full[0:n_nodes, :].opt()],
            )
            nc.sync.dma_start(out=h1full[n_nodes : n_nodes + 1, :], in_=zrow[0:1, :])

            # ---- 3. layer-1 gather + aggregate ----
            t = 0
            for w in range(wn):
                pw = pa.tile([hid, 128], F32, tag="pagg")
                for k in range(k_w[w]):
                    g = gp.tile([128, hid], F32, tag="g")
                    nc.gpsimd.indirect_dma_start(
                        out=g[:],
                        out_offset=None,
                        in_=h1full[:],
                        in_offset=IndirectOffsetOnAxis(ap=idx_t[:, t : t + 1], axis=0),
                    )
                    s = sp.tile([128, 128], F32, tag="s")
                    nc.vector.tensor_scalar(
                        out=s[:],
                        in0=iota_t[:],
                        scalar1=drel_t[:, t : t + 1],
                        scalar2=nrm_t[:, t : t + 1],
                        op0=eq,
                        op1=mul,
                    )
                    nc.tensor.matmul(
                        out=pw[:],
                        lhsT=g[:],
                        rhs=s[:],
                        start=(k == 0),
                        stop=(k == k_w[w] - 1),
                    )
                    t += 1
                # epilogue: acc1T[:, w*128: ] = relu(pw + b1)
                nc.scalar.activation(
                    out=acc1T[:, w * 128 : (w + 1) * 128],
                    in_=pw[:],
                    func=mybir.ActivationFunctionType.Relu,
                    bias=b1_t[:],
                )

            # ---- 4. h2pre = relu_out @ W2 ----
            for b in range(nb):
                cols = min(128, nloc - b * 128)
                ps2 = pm.tile([128, hid], F32, tag="pmm")
                nc.tensor.matmul(
                    out=ps2[:cols, :],
                    lhsT=acc1T[:, b * 128 : b * 128 + cols],
                    rhs=w2_t[:],
                    start=True,
                    stop=True,
                )
                hb2 = ep.tile([128, hid], F32, tag="hb")
                nc.vector.tensor_copy(out=hb2[:cols, :], in_=ps2[:cols, :])
                nc.sync.dma_start(
                    out=h2loc[b * 128 : b * 128 + cols, :], in_=hb2[:cols, :]
                )

            nc.gpsimd.collective_compute(
                "AllGather",
                mybir.AluOpType.bypass,
                replica_groups=[list(range(n_cores))],
                ins=[h2loc[:].opt()],
                outs=[h2full[0:n_nodes, :].opt()],
            )
            nc.sync.dma_start(out=h2full[n_nodes : n_nodes + 1, :], in_=zrow[0:1, :])

            # ---- 5. layer-2 gather + aggregate + log_softmax ----
            t = 0
            for w in range(wn):
                rows = min(128, nloc - w * 128)
                po = pa.tile([128, hid], F32, tag="pagg")
                for k in range(k_w[w]):
                    g2 = gp.tile([128, hid], F32, tag="g")
                    nc.gpsimd.indirect_dma_start(
                        out=g2[:],
                        out_offset=None,
                        in_=h2full[:],
                        in_offset=IndirectOffsetOnAxis(ap=idx_t[:, t : t + 1], axis=0),
                    )
                    s2 = sp.tile([128, 128], F32, tag="s")
                    nc.vector.tensor_scalar(
                        out=s2[:],
                        in0=iota_t[:],
                        scalar1=drel_t[:, t : t + 1],
                        scalar2=nrm_t[:, t : t + 1],
                        op0=eq,
                        op1=mul,
                    )
                    nc.tensor.matmul(
                        out=po[:],
                        lhsT=s2[:],
                        rhs=g2[:],
                        start=(k == 0),
                        stop=(k == k_w[w] - 1),
                    )
                    t += 1
                zt = ep.tile([128, hid], F32, tag="zt")
                nc.vector.tensor_tensor(out=zt[:], in0=po[:], in1=b2_t[:], op=mybir.AluOpType.add)
                mx = ep.tile([128, 1], F32, tag="mx")
                nc.vector.reduce_max(mx[:], zt[:, :out_dim], axis=mybir.AxisListType.X)
                sh = ep.tile([128, out_dim], F32, tag="sh")
                nc.vector.tensor_scalar_sub(out=sh[:], in0=zt[:, :out_dim], scalar1=mx[:])
                ex = ep.tile([128, out_dim], F32, tag="ex")
                sm = ep.tile([128, 1], F32, tag="sm")
                nc.scalar.activation(
                    out=ex[:], in_=sh[:], func=mybir.ActivationFunctionType.Exp,
                    accum_out=sm[:],
                )
                lnt = ep.tile([128, 1], F32, tag="lnt")
                nc.scalar.activation(
                    out=lnt[:], in_=sm[:], func=mybir.ActivationFunctionType.Ln
                )
                res = ep.tile([128, out_dim], F32, tag="res")
                nc.vector.tensor_scalar_sub(out=res[:], in0=sh[:], scalar1=lnt[:])
                nc.sync.dma_start(
                    out=out[w * 128 : w * 128 + rows, :], in_=res[:rows, :]
                )

    _split_long_waits(nc)
    return nc


def _kernel_impl(x, edge_index, W1, b1, W2, b2, n_nodes, n_cores):
    x = np.asarray(x, dtype=np.float32)
    W1 = np.asarray(W1, dtype=np.float32)
    b1 = np.asarray(b1, dtype=np.float32)
    W2 = np.asarray(W2, dtype=np.float32)
    b2 = np.asarray(b2, dtype=np.float32)

    in_dim = x.shape[1]
    hid = W1.shape[1]
    out_dim = W2.shape[1]

    meta = _preprocess(edge_index, n_nodes, n_cores)
    nloc, wn, k_w, t_total = meta["nloc"], meta["wn"], meta["k_w"], meta["t_total"]

    nc = _build_nc(n_nodes, nloc, wn, k_w, t_total, hid, out_dim, in_dim, n_cores)

    w2p = np.zeros((hid, hid), dtype=np.float32)
    w2p[:, :out_dim] = W2
    b2p = np.zeros((hid,), dtype=np.float32)
    b2p[:out_dim] = b2
    b2rep = np.tile(b2p[None, :], (128, 1)).copy()
    b1c = b1.reshape(hid, 1).copy()
    iota = np.tile(np.arange(128, dtype=np.float32)[None, :], (128, 1)).copy()

    in_maps = []
    for c in range(n_cores):
        xs = x[c * nloc : (c + 1) * nloc]
        in_maps.append(
            {
                "xT": np.ascontiguousarray(xs.T),
                "idx": meta["idx"][c],
                "drel": meta["drel"][c],
                "nrm": meta["norm"][c],
                "w1": W1,
                "w2p": w2p,
                "b1c": b1c,
                "b2rep": b2rep,
                "iota": iota,
            }
        )

    res = run_bass_kernel_spmd(nc, in_maps, core_ids=list(range(n_cores)))
    return np.concatenate([res.results[c]["out"] for c in range(n_cores)], axis=0)


def kernel(x, edge_index, W1, b1, W2, b2):
    return _kernel_impl(x, edge_index, W1, b1, W2, b2, N_NODES, N_CORES)


# revision 15
# speedup vs baseline: 4.8045x; 4.8045x over previous
"""Two-layer GCN (PyG GCNConv semantics) on 8 Trainium2 NeuronCores.

Strategy (sharding_hint): nodes are sharded row-wise across the 8 cores;
edges are partitioned by destination node so the segment-sum stays local;
source-node features are exchanged with an on-device AllGather between
layers; the small weight matrices are replicated.

Device pipeline per core (single NEFF, statically unrolled, SPMD-uniform):
  1. h1pre = x @ W1 for the local node shard (PE), written to DRAM.
  2. AllGather h1pre -> full gather table [N, HID].
  3. Per 128-node destination window: indirect-DMA gather of source rows,
     build a norm-weighted one-hot selection matrix on DVE
     (S[e, j] = norm[e] * (dst_rel[e] == j)), and accumulate
     msgs^T @ S into PSUM (PE).  Epilogue: relu(. + b1) (ACT) into an
     SBUF-resident transposed activation accumulator.
  4. h2pre = relu_out @ W2 (PE), AllGather again.
  5. Second gather/aggregate pass (S as lhsT so nodes land on partitions),
     then bias + log_softmax (DVE reduce + ACT exp/ln) and DMA out.

Edge bookkeeping (sorting by window, slot assignment, padding so that all
8 cores share one instruction stream) is host-side numpy index work; all
floating-point math on features runs on device.
"""

import math

import numpy as np

import concourse.bass as bass
import concourse.mybir as mybir
import concourse.tile as tile
from concourse.bass import IndirectOffsetOnAxis
from concourse.bass_utils import run_bass_kernel_spmd

N_NODES = 100000
N_EDGES = 1600000
IN_DIM, HID_DIM, OUT_DIM = 128, 64, 40
N_CORES = 8

F32 = mybir.dt.float32
I32 = mybir.dt.int32

# Engine for building the one-hot selection matrices. "act" keeps the vector
# engine quiet so its 2-port SBUF perf mode never locks GPSIMD (SWDGE
# descriptor rings) out of SBUF while gather descriptors are being emitted.
S_ENGINE = "act"
# Diagnostics (leave False for correct results): skip S-builds / aggregation
# matmuls to isolate the gather pipeline when profiling.
DIAG_NO_SBUILD = False
DIAG_NO_MM = False


def _split_long_waits(nc, max_waits=1):
    """This toolchain's codegen rejects instructions carrying more than one
    semaphore wait; move extra waits onto preceding same-engine no-ops."""
    cnt = 0
    for bb in nc.main_func.blocks:
        i = 0
        insts = bb.instructions
        while i < len(insts):
            ins = insts[i]
            si = ins.sync_info
            if si is not None and si.on_wait and len(si.on_wait) > max_waits:
                waits = list(si.on_wait)
                keep = waits[-max_waits:]
                extra = waits[:-max_waits]
                si.on_wait = keep
                new_insts = []
                for j in range(0, len(extra), max_waits):
                    chunk = extra[j : j + max_waits]
                    nop = mybir.InstNoOp(
                        name=f"{ins.name}-waitsplit-{j}",
                        engine=ins.engine,
                        ins=[],
                        outs=[],
                        sync_info=mybir.SyncInfo(on_wait=chunk, on_update=[]),
                    )
                    new_insts.append(nop)
                insts[i:i] = new_insts
                i += len(new_insts)
                cnt += len(new_insts)
            i += 1
    return cnt


def _preprocess(edge_index, n_nodes, n_cores):
    """Host-side index bookkeeping. Returns per-core slot arrays + layout."""
    nloc = n_nodes // n_cores
    wn = math.ceil(nloc / 128)

    src = np.asarray(edge_index[0], dtype=np.int64)
    dst = np.asarray(edge_index[1], dtype=np.int64)
    loop = np.arange(n_nodes, dtype=np.int64)
    src_all = np.concatenate([src, loop])
    dst_all = np.concatenate([dst, loop])

    deg = np.bincount(dst_all, minlength=n_nodes).astype(np.float64)
    dis = np.where(deg > 0, 1.0 / np.sqrt(deg), 0.0)
    norm = (dis[src_all] * dis[dst_all]).astype(np.float32)

    core = dst_all // nloc
    dloc = dst_all - core * nloc
    w = dloc >> 7
    dst_rel = (dloc & 127).astype(np.float32)

    key = core * wn + w
    order = np.argsort(key, kind="stable")
    counts = np.bincount(key, minlength=n_cores * wn).reshape(n_cores, wn)

    k_w = np.maximum((counts + 127) // 128, 1).max(axis=0)  # tiles per window
    t_total = int(k_w.sum())
    tile_base = np.zeros(wn, dtype=np.int64)
    tile_base[1:] = np.cumsum(k_w)[:-1]

    # rank of each edge within its (core, window) group, in sorted order
    grp_start = np.zeros(n_cores * wn, dtype=np.int64)
    grp_start[1:] = np.cumsum(counts.reshape(-1))[:-1]
    rank = np.arange(len(order), dtype=np.int64) - grp_start[key[order]]

    slot = (tile_base[w[order]] * 128 + rank).astype(np.int64)

    n_slots = t_total * 128
    idx_a = np.full((n_cores, n_slots), n_nodes, dtype=np.int32)
    drel_a = np.zeros((n_cores, n_slots), dtype=np.float32)
    norm_a = np.zeros((n_cores, n_slots), dtype=np.float32)

    c_o = core[order]
    idx_a[c_o, slot] = src_all[order].astype(np.int32)
    drel_a[c_o, slot] = dst_rel[order]
    norm_a[c_o, slot] = norm[order]

    # device layout: [128, T] with slot s at (s % 128, s // 128)
    idx_dev = [idx_a[c].reshape(t_total, 128).T.copy() for c in range(n_cores)]
    drel_dev = [drel_a[c].reshape(t_total, 128).T.copy() for c in range(n_cores)]
    norm_dev = [norm_a[c].reshape(t_total, 128).T.copy() for c in range(n_cores)]

    return {
        "nloc": nloc,
        "wn": wn,
        "k_w": [int(v) for v in k_w],
        "t_total": t_total,
        "idx": idx_dev,
        "drel": drel_dev,
        "norm": norm_dev,
    }


def _build_nc(n_nodes, nloc, wn, k_w, t_total, hid, out_dim, in_dim, n_cores):
    nc = bass.Bass(num_devices=n_cores)

    xT = nc.dram_tensor("xT", [in_dim, nloc], F32, kind="ExternalInput")
    idx = nc.dram_tensor("idx", [128, t_total], I32, kind="ExternalInput")
    drel = nc.dram_tensor("drel", [128, t_total], F32, kind="ExternalInput")
    nrm = nc.dram_tensor("nrm", [128, t_total], F32, kind="ExternalInput")
    nneg = nc.dram_tensor("nneg", [128, t_total], F32, kind="ExternalInput")
    w1 = nc.dram_tensor("w1", [in_dim, hid], F32, kind="ExternalInput")
    w2p = nc.dram_tensor("w2p", [hid, hid], F32, kind="ExternalInput")
    b1c = nc.dram_tensor("b1c", [hid, 1], F32, kind="ExternalInput")
    b2rep = nc.dram_tensor("b2rep", [128, hid], F32, kind="ExternalInput")
    iota_in = nc.dram_tensor("iota", [128, 128], F32, kind="ExternalInput")
    out = nc.dram_tensor("out", [nloc, out_dim], F32, kind="ExternalOutput")

    nb = math.ceil(nloc / 128)
    eq = mybir.AluOpType.is_equal
    mul = mybir.AluOpType.mult
    sub = mybir.AluOpType.subtract

    with tile.TileContext(nc) as tc:
        with (
            tc.tile_pool(name="const", bufs=1) as cp,
            tc.tile_pool(name="gpool", bufs=16) as gp,
            tc.tile_pool(name="spool", bufs=8) as sp,
            tc.tile_pool(name="evac", bufs=4) as ep,
            tc.tile_pool(name="ps_agg", bufs=4, space="PSUM") as pa,
            tc.tile_pool(name="ps_mm", bufs=4, space="PSUM") as pm,
            tc.tile_pool(name="dram", bufs=1, space="DRAM") as dp,
        ):
            # ---- resident tensors ----
            xT_t = cp.tile([in_dim, nloc], F32)
            nc.sync.dma_start(out=xT_t[:], in_=xT[:])
            idx_t = cp.tile([128, t_total], I32)
            nc.sync.dma_start(out=idx_t[:], in_=idx[:])
            drel_t = cp.tile([128, t_total], F32)
            nc.sync.dma_start(out=drel_t[:], in_=drel[:])
            nrm_t = cp.tile([128, t_total], F32)
            nc.sync.dma_start(out=nrm_t[:], in_=nrm[:])
            nneg_t = cp.tile([128, t_total], F32)
            nc.sync.dma_start(out=nneg_t[:], in_=nneg[:])
            w1_t = cp.tile([in_dim, hid], F32)
            nc.sync.dma_start(out=w1_t[:], in_=w1[:])
            w2_t = cp.tile([hid, hid], F32)
            nc.sync.dma_start(out=w2_t[:], in_=w2p[:])
            b1_t = cp.tile([hid, 1], F32)
            nc.sync.dma_start(out=b1_t[:], in_=b1c[:])
            b2_t = cp.tile([128, hid], F32)
            nc.sync.dma_start(out=b2_t[:], in_=b2rep[:])
            iota_t = cp.tile([128, 128], F32)
            nc.sync.dma_start(out=iota_t[:], in_=iota_in[:])
            acc1T = cp.tile([hid, wn * 128], F32)
            zrow = cp.tile([128, hid], F32)
            nc.vector.memset(zrow[:], 0.0)

            h1loc = dp.tile([nloc, hid], F32)
            h1full = dp.tile([n_nodes + 128, hid], F32)
            h2loc = dp.tile([nloc, hid], F32)
            h2full = dp.tile([n_nodes + 128, hid], F32)

            s_const = cp.tile([128, 128], F32)
            nc.vector.memset(s_const[:], 0.0)

            def build_s(t):
                """S[e, j] = norm[e] * (dst_rel[e] == j), exact in f32."""
                if DIAG_NO_SBUILD:
                    return s_const
                s = sp.tile([128, 128], F32, tag="s", name="s")
                if S_ENGINE == "act":
                    tmp = sp.tile([128, 128], F32, tag="stmp", name="stmp")
                    # (drel - iota)^2
                    nc.scalar.activation(
                        out=tmp[:], in_=iota_t[:],
                        func=mybir.ActivationFunctionType.Square,
                        bias=drel_t[:, t : t + 1], scale=-1.0,
                    )
                    # relu(norm - norm * t2) -> norm iff t2 == 0
                    nc.scalar.activation(
                        out=s[:], in_=tmp[:],
                        func=mybir.ActivationFunctionType.Relu,
                        bias=nrm_t[:, t : t + 1], scale=nneg_t[:, t : t + 1],
                    )
                else:
                    nc.vector.tensor_scalar(
                        out=s[:], in0=iota_t[:],
                        scalar1=drel_t[:, t : t + 1],
                        scalar2=nrm_t[:, t : t + 1],
                        op0=mybir.AluOpType.is_equal, op1=mybir.AluOpType.mult,
                    )
                return s

            # ---- 1. h1pre = x @ W1 (local shard) ----
            for b in range(nb):
                cols = min(128, nloc - b * 128)
                ps = pm.tile([128, hid], F32, tag="pmm")
                nc.tensor.matmul(
                    out=ps[:cols, :],
                    lhsT=xT_t[:, b * 128 : b * 128 + cols],
                    rhs=w1_t[:],
                    start=True,
                    stop=True,
                )
                hb = ep.tile([128, hid], F32, tag="hb")
                nc.vector.tensor_copy(out=hb[:cols, :], in_=ps[:cols, :])
                nc.sync.dma_start(
                    out=h1loc[b * 128 : b * 128 + cols, :], in_=hb[:cols, :]
                )

            # ---- 2. AllGather + zero pad row ----
            nc.gpsimd.collective_compute(
                "AllGather",
                mybir.AluOpType.bypass,
                replica_groups=[list(range(n_cores))],
                ins=[h1loc[:].opt()],
                outs=[h1full[0:n_nodes, :].opt()],
            )
            nc.sync.dma_start(out=h1full[n_nodes : n_nodes + 1, :], in_=zrow[0:1, :])

            # ---- 3. layer-1 gather + aggregate ----
            t = 0
            for w in range(wn):
                pw = pa.tile([hid, 128], F32, tag="pagg")
                for k in range(k_w[w]):
                    g = gp.tile([128, hid], F32, tag="g")
                    nc.gpsimd.indirect_dma_start(
                        out=g[:],
                        out_offset=None,
                        in_=h1full[:],
                        in_offset=IndirectOffsetOnAxis(ap=idx_t[:, t : t + 1], axis=0),
                    )
                    s = build_s(t)
                    nc.tensor.matmul(
                        out=pw[:],
                        lhsT=g[:],
                        rhs=s[:],
                        start=(k == 0),
                        stop=(k == k_w[w] - 1),
                    )
                    t += 1
                # epilogue: acc1T[:, w*128: ] = relu(pw + b1)
                nc.scalar.activation(
                    out=acc1T[:, w * 128 : (w + 1) * 128],
                    in_=pw[:],
                    func=mybir.ActivationFunctionType.Relu,
                    bias=b1_t[:],
                )

            # ---- 4. h2pre = relu_out @ W2 ----
            for b in range(nb):
                cols = min(128, nloc - b * 128)
                ps2 = pm.tile([128, hid], F32, tag="pmm")
                nc.tensor.matmul(
                    out=ps2[:cols, :],
                    lhsT=acc1T[:, b * 128 : b * 128 + cols],
                    rhs=w2_t[:],
                    start=True,
                    stop=True,
                )
                hb2 = ep.tile([128, hid], F32, tag="hb")
                nc.vector.tensor_copy(out=hb2[:cols, :], in_=ps2[:cols, :])
                nc.sync.dma_start(
                    out=h2loc[b * 128 : b * 128 + cols, :], in_=hb2[:cols, :]
                )

            nc.gpsimd.collective_compute(
                "AllGather",
                mybir.AluOpType.bypass,
                replica_groups=[list(range(n_cores))],
                ins=[h2loc[:].opt()],
                outs=[h2full[0:n_nodes, :].opt()],
            )
            nc.sync.dma_start(out=h2full[n_nodes : n_nodes + 1, :], in_=zrow[0:1, :])

            # ---- 5. layer-2 gather + aggregate + log_softmax ----
            t = 0
            for w in range(wn):
                rows = min(128, nloc - w * 128)
                po = pa.tile([128, hid], F32, tag="pagg")
                for k in range(k_w[w]):
                    g2 = gp.tile([128, hid], F32, tag="g")
                    nc.gpsimd.indirect_dma_start(
                        out=g2[:],
                        out_offset=None,
                        in_=h2full[:],
                        in_offset=IndirectOffsetOnAxis(ap=idx_t[:, t : t + 1], axis=0),
                    )
                    s2 = build_s(t)
                    nc.tensor.matmul(
                        out=po[:],
                        lhsT=s2[:],
                        rhs=g2[:],
                        start=(k == 0),
                        stop=(k == k_w[w] - 1),
                    )
                    t += 1
                zt = ep.tile([128, hid], F32, tag="zt")
                nc.vector.tensor_tensor(out=zt[:], in0=po[:], in1=b2_t[:], op=mybir.AluOpType.add)
                mx = ep.tile([128, 1], F32, tag="mx")
                nc.vector.reduce_max(mx[:], zt[:, :out_dim], axis=mybir.AxisListType.X)
                sh = ep.tile([128, out_dim], F32, tag="sh")
                nc.vector.tensor_scalar_sub(out=sh[:], in0=zt[:, :out_dim], scalar1=mx[:])
                ex = ep.tile([128, out_dim], F32, tag="ex")
                sm = ep.tile([128, 1], F32, tag="sm")
                nc.scalar.activation(
                    out=ex[:], in_=sh[:], func=mybir.ActivationFunctionType.Exp,
                    accum_out=sm[:],
                )
                lnt = ep.tile([128, 1], F32, tag="lnt")
                nc.scalar.activation(
                    out=lnt[:], in_=sm[:], func=mybir.ActivationFunctionType.Ln
                )
                res = ep.tile([128, out_dim], F32, tag="res")
                nc.vector.tensor_scalar_sub(out=res[:], in0=sh[:], scalar1=lnt[:])
                nc.sync.dma_start(
                    out=out[w * 128 : w * 128 + rows, :], in_=res[:rows, :]
                )

    _split_long_waits(nc)
    return nc


def _kernel_impl(x, edge_index, W1, b1, W2, b2, n_nodes, n_cores):
    x = np.asarray(x, dtype=np.float32)
    W1 = np.asarray(W1, dtype=np.float32)
    b1 = np.asarray(b1, dtype=np.float32)
    W2 = np.asarray(W2, dtype=np.float32)
    b2 = np.asarray(b2, dtype=np.float32)

    in_dim = x.shape[1]
    hid = W1.shape[1]
    out_dim = W2.shape[1]

    meta = _preprocess(edge_index, n_nodes, n_cores)
    nloc, wn, k_w, t_total = meta["nloc"], meta["wn"], meta["k_w"], meta["t_total"]

    nc = _build_nc(n_nodes, nloc, wn, k_w, t_total, hid, out_dim, in_dim, n_cores)

    w2p = np.zeros((hid, hid), dtype=np.float32)
    w2p[:, :out_dim] = W2
    b2p = np.zeros((hid,), dtype=np.float32)
    b2p[:out_dim] = b2
    b2rep = np.tile(b2p[None, :], (128, 1)).copy()
    b1c = b1.reshape(hid, 1).copy()
    iota = np.tile(np.arange(128, dtype=np.float32)[None, :], (128, 1)).copy()

    in_maps = []
    for c in range(n_cores):
        xs = x[c * nloc : (c + 1) * nloc]
        in_maps.append(
            {
                "xT": np.ascontiguousarray(xs.T),
                "idx": meta["idx"][c],
                "drel": meta["drel"][c],
                "nrm": meta["norm"][c],
                "nneg": -meta["norm"][c],
                "w1": W1,
                "w2p": w2p,
                "b1c": b1c,
                "b2rep": b2rep,
                "iota": iota,
            }
        )

    res = run_bass_kernel_spmd(nc, in_maps, core_ids=list(range(n_cores)))
    return np.concatenate([res.results[c]["out"] for c in range(n_cores)], axis=0)


def kernel(x, edge_index, W1, b1, W2, b2):
    return _kernel_impl(x, edge_index, W1, b1, W2, b2, N_NODES, N_CORES)
